# revision 50
# baseline (speedup 1.0000x reference)
"""MiniMaxText01 lightning-attention kernel for 8 TRN2 NeuronCores.

Sharding: 8 cores = 2 batches x 4 sequence quarters (token-parallel).
Each core runs the whole pipeline (qkv proj -> decay block scan -> RMSNorm
-> gate -> out proj) for its 1024 tokens; the only cross-core data is the
kv-state prefix, exchanged via a 1MB AllGather of per-core decayed kv
contributions within each batch's 4-core group.

All matmuls in bf16 (fp32 PSUM accumulation); RMS scale path in fp32.
v2: weight DMAs split across sync+scalar HWDGE queues, RMS sums fused
into the block scan, gate GEMM pipelined across 4-head groups, out-proj
with fully-resident Wo and 1MB stores.
"""

import sys

sys.path.insert(0, "/opt/trn_rl_repo")

import ml_dtypes
import numpy as np

import types

try:
    import antenv.axon_hooks  # noqa: F401
except ImportError:
    try:
        import antenv
        from trn_agent_boot.trn_boot import _ntff_profile_via_ctypes

        _m = types.ModuleType("antenv.axon_hooks")
        _m._hook = _ntff_profile_via_ctypes("/opt/axon/libaxon_pjrt.so")
        _m.get_axon_ntff_profile_hook = lambda: _m._hook
        _m.set_axon_ntff_profile_hook = lambda h: setattr(_m, "_hook", h)
        sys.modules["antenv.axon_hooks"] = _m
        antenv.axon_hooks = _m
    except Exception:
        pass

import concourse.bass as bass
import concourse.mybir as mybir
from concourse import bacc
from concourse.tile import TileContext
from concourse.bass_utils import run_bass_kernel_spmd

BF16 = mybir.dt.bfloat16
FP32 = mybir.dt.float32
AF = mybir.ActivationFunctionType
OP = mybir.AluOpType
bf16 = ml_dtypes.bfloat16

B, N, HID = 2, 4096, 2048
H, D, BLK = 16, 128, 256
T = 1024          # tokens per core
NBLK = T // BLK   # 4 local blocks
KC = HID // 128   # 16 contraction chunks
NC = 8
EPS = float(np.finfo(np.float32).eps)


def _build(bd):
    """Build the SPMD bass program. bd: (16,) python floats exp(-256*s_h)."""
    nc = bacc.Bacc("TRN2", target_bir_lowering=False, debug=False, num_devices=NC)

    xT_d = nc.dram_tensor("xT", [HID, T], BF16, kind="ExternalInput")
    wqk_d = nc.dram_tensor("wqkT", [HID, H * 2 * D], BF16, kind="ExternalInput")
    wv_d = nc.dram_tensor("wvT", [HID, H * D], BF16, kind="ExternalInput")
    c_d = nc.dram_tensor("cscratch", [H * NBLK * 128, 128], BF16, kind="Internal")
    wgT_d = nc.dram_tensor("wgT", [HID, H * D], BF16, kind="ExternalInput")
    woT_d = nc.dram_tensor("woT", [H * D, HID], BF16, kind="ExternalInput")
    maskq_d = nc.dram_tensor("maskq", [H * 128, 3 * BLK], BF16, kind="ExternalInput")
    kdec_d = nc.dram_tensor("kdec", [128, H * 2], FP32, kind="ExternalInput")
    wrep_d = nc.dram_tensor("wrep", [128, H * 4], FP32, kind="ExternalInput")
    ident_d = nc.dram_tensor("ident", [128, 128], BF16, kind="ExternalInput")
    ones_d = nc.dram_tensor("ones", [128, 1], BF16, kind="ExternalInput")
    onesf_d = nc.dram_tensor("onesf", [1, 128], FP32, kind="ExternalInput")
    out_d = nc.dram_tensor("out", [T, HID], BF16, kind="ExternalOutput")
    ccin_g = [
        nc.dram_tensor(f"ccin{g}", [4 * 128, 128], FP32, kind="Internal")
        for g in range(4)
    ]
    ccout_g = [
        nc.dram_tensor(f"ccout{g}", [4 * 4 * 128, 128], FP32, kind="Internal")
        for g in range(4)
    ]
    warm_in = nc.dram_tensor("warmin", [128, 128], FP32, kind="Internal")
    warm_out = nc.dram_tensor("warmout", [4 * 128, 128], FP32, kind="Internal")

    with TileContext(nc) as tc:
        with (
            tc.tile_pool(name="const", bufs=1) as cp,
            tc.tile_pool(name="persist", bufs=1) as pp,
            tc.tile_pool(name="work", bufs=2) as wp,
        ):
            # warm up the collective path so the first real AllGather is hot
            nc.gpsimd.collective_compute(
                "AllGather",
                OP.bypass,
                ins=[warm_in[:]],
                outs=[warm_out[:]],
                replica_groups=[[0, 1, 2, 3], [4, 5, 6, 7]],
            )
            ident = cp.tile([128, 128], BF16)
            nc.scalar.dma_start(ident[:], ident_d[:])
            onesb = cp.tile([128, 1], BF16)
            nc.scalar.dma_start(onesb[:], ones_d[:])
            kdec = cp.tile([128, H * 2], FP32)
            nc.scalar.dma_start(kdec[:], kdec_d[:])
            wrep = cp.tile([128, H * 4], FP32)
            nc.scalar.dma_start(wrep[:], wrep_d[:])
            onesf = cp.tile([1, 128], FP32)
            nc.scalar.dma_start(onesf[:], onesf_d[:])
            sc = cp.tile([128, T], BF16)

            qT, kT, outT = [], [], []
            for h in range(H):
                outT.append(pp.tile([128, T], BF16, tag=f"o{h}", name=f"outT{h}"))

            # ---------------- PASS A: qkv projection + kv contributions
            with tc.tile_pool(name="qkv", bufs=1) as qp:
                with tc.tile_pool(name="xTA", bufs=1) as xp:
                    xk = []
                    for kk in range(KC):
                        xt = xp.tile([128, T], BF16, tag=f"x{kk}")
                        nc.gpsimd.dma_start(xt[:], xT_d[kk * 128 : (kk + 1) * 128, :])
                        xk.append(xt)
                    # v projection, token-partition orientation, one 512-col
                    # group at a time (keeps resident v-weights at 1MB)
                    vn = [
                        qp.tile([128, H * D], BF16, tag=f"vn{tt}", name=f"vn{tt}")
                        for tt in range(T // 128)
                    ]
                    with tc.tile_pool(name="wvp", bufs=1) as wvpool, \
                         tc.tile_pool(name="psV", bufs=1, space="PSUM") as psV:
                        for oc in range(4):
                            pvs = {}
                            for kkh in range(2):
                                wv_tiles = []
                                for k8 in range(8):
                                    kk = kkh * 8 + k8
                                    wvp = wvpool.tile(
                                        [128, 512], BF16, tag=f"wvk{kk}",
                                        name=f"wvk{kk}_{oc}",
                                    )
                                    eng = nc.sync if k8 % 2 == 0 else nc.scalar
                                    eng.dma_start(
                                        wvp[:],
                                        wv_d[
                                            kk * 128 : (kk + 1) * 128,
                                            oc * 512 : (oc + 1) * 512,
                                        ],
                                    )
                                    wv_tiles.append(wvp)
                                for tt in range(T // 128):
                                    if kkh == 0:
                                        pvs[tt] = psV.tile(
                                            [128, 512], FP32, tag=f"vo{tt}",
                                            name=f"pv{oc}_{tt}",
                                        )
                                    pv = pvs[tt]
                                    for k8 in range(8):
                                        kk = kkh * 8 + k8
                                        nc.tensor.matmul(
                                            pv[:],
                                            xk[kk][:, tt * 128 : (tt + 1) * 128],
                                            wv_tiles[k8][:],
                                            start=(kk == 0), stop=(kk == KC - 1),
                                        )
                                    if kkh == 1:
                                        nc.scalar.activation(
                                            vn[tt][:, oc * 512 : (oc + 1) * 512],
                                            pv[:], AF.Silu,
                                        )
                    with tc.tile_pool(name="wqkp", bufs=1) as wqkp, tc.tile_pool(
                        name="psA", bufs=1, space="PSUM"
                    ) as psA, tc.tile_pool(name="psT", bufs=1, space="PSUM") as psT:
                        for h in range(H):
                            q_t = qp.tile([128, T], BF16, tag=f"q{h}", name=f"q{h}")
                            k_t = qp.tile([128, T], BF16, tag=f"k{h}", name=f"k{h}")
                            qT.append(q_t)
                            kT.append(k_t)
                            ps = [
                                psA.tile([128, 512], FP32, tag=f"pj{i}",
                                         name=f"pj{h}_{i}")
                                for i in range(4)
                            ]
                            wts = []
                            for kk in range(KC):
                                wt = wqkp.tile([128, 2 * D], BF16, tag=f"w{kk}",
                                               name=f"w{h}_{kk}")
                                eng = nc.sync if kk % 2 == 0 else nc.scalar
                                eng.dma_start(
                                    wt[:],
                                    wqk_d[
                                        kk * 128 : (kk + 1) * 128,
                                        h * 2 * D : (h + 1) * 2 * D,
                                    ],
                                )
                                wts.append(wt)
                            for kk in range(KC):
                                for si in range(2):
                                    lhs = wts[kk][:, si * 128 : (si + 1) * 128]
                                    for nn in range(2):
                                        nc.tensor.matmul(
                                            ps[2 * si + nn][:],
                                            lhs,
                                            xk[kk][:, nn * 512 : (nn + 1) * 512],
                                            start=(kk == 0),
                                            stop=(kk == KC - 1),
                                        )
                            for si, dst in enumerate((q_t, k_t)):
                                for nn in range(2):
                                    nc.scalar.activation(
                                        dst[:, nn * 512 : (nn + 1) * 512],
                                        ps[2 * si + nn][:],
                                        AF.Silu,
                                    )
                            # kv contributions of local blocks, decayed to core end
                            totB = wp.tile([128, 128], FP32, tag="totB")
                            for j in range(NBLK):
                                csum = psT.tile([128, 128], FP32, tag="Cp")
                                for hf in range(2):
                                    col = j * BLK + hf * 128
                                    tt = j * 2 + hf
                                    pt = psT.tile([128, 128], BF16, tag="tr")
                                    nc.tensor.transpose(
                                        pt[:], k_t[:, col : col + 128], ident[:]
                                    )
                                    ks = wp.tile([128, 128], BF16, tag="ks")
                                    nc.vector.tensor_scalar_mul(
                                        ks[:], pt[:],
                                        kdec[:, 2 * h + hf : 2 * h + hf + 1],
                                    )
                                    nc.tensor.matmul(
                                        csum[:], ks[:],
                                        vn[tt][:, h * 128 : (h + 1) * 128],
                                        start=(hf == 0), stop=(hf == 1),
                                    )
                                if j < NBLK - 1:
                                    cb = wp.tile([128, 128], BF16, tag="cb",
                                                 bufs=6)
                                    nc.vector.tensor_copy(cb[:], csum[:])
                                    nc.gpsimd.dma_start(
                                        c_d[(h * NBLK + j) * 128
                                            : (h * NBLK + j + 1) * 128, :],
                                        cb[:],
                                    )
                                w = bd[h] ** (NBLK - 1 - j)
                                if j == 0:
                                    nc.vector.tensor_scalar_mul(totB[:], csum[:], w)
                                else:
                                    nc.vector.scalar_tensor_tensor(
                                        totB[:], csum[:], w, totB[:], OP.mult, OP.add
                                    )
                            g, hg = h // 4, h % 4
                            nc.sync.dma_start(
                                ccin_g[g][hg * 128 : (hg + 1) * 128, :], totB[:]
                            )
                            if hg == 3:
                                nc.gpsimd.collective_compute(
                                    "AllGather",
                                    OP.bypass,
                                    ins=[ccin_g[g][:]],
                                    outs=[ccout_g[g][:]],
                                    replica_groups=[[0, 1, 2, 3], [4, 5, 6, 7]],
                                )
                            if h == H - 1:
                                # WAW anchor: bind the ccout-load tile tags to
                                # end-of-pass-A work so the scheduler cannot
                                # queue those (collective-gated) DMAs ahead of
                                # the remaining c_d stores on the same queue.
                                for tgi in range(2):
                                    gw = wp.tile([128, 128], FP32, tag=f"g{tgi}")
                                    nc.vector.tensor_copy(gw[:], totB[:])

                def entering_kv(h):
                    """kv state entering this core for head h, from the AllGather."""
                    ent = wp.tile([128, 128], FP32, tag="ent")
                    gi, hg = h // 4, h % 4
                    for p in range(4):
                        g = wp.tile([128, 128], FP32, tag=f"g{p % 2}")
                        nc.gpsimd.dma_start(
                            g[:],
                            ccout_g[gi][
                                (p * 4 + hg) * 128 : (p * 4 + hg + 1) * 128, :
                            ],
                        )
                        if p == 0:
                            nc.vector.tensor_scalar_mul(
                                ent[:], g[:], wrep[:, 4 * h : 4 * h + 1]
                            )
                        else:
                            nc.vector.scalar_tensor_tensor(
                                ent[:], g[:], wrep[:, 4 * h + p : 4 * h + p + 1],
                                ent[:], OP.mult, OP.add,
                            )
                    kv = pp.tile([128, 128], BF16, tag=f"kv{h}b")
                    nc.vector.tensor_copy(kv[:], ent[:])
                    return kv

                # ---------------- PASS B: decay block scan (+ fused RMS sums)
                # B1 (per head): dense intra-block attention, no serial deps.
                # B2 (per 4-head group): inter-block kv chains, 4 chains
                # interleaved so the PE never waits on one chain's latency.
                with tc.tile_pool(name="psS", bufs=1, space="PSUM") as psS:
                    s0 = psS.tile([1, 512], FP32, tag="s0")
                    s1 = psS.tile([1, 512], FP32, tag="s1")
                    with tc.tile_pool(name="psB", bufs=2, space="PSUM") as psB, \
                         tc.tile_pool(name="mp", bufs=2) as mp:
                        for hgi in range(4):
                            heads = range(4 * hgi, 4 * hgi + 4)
                            # ---- vector-side prep (off the PE critical path):
                            # local kv states from the c_d block contributions
                            # (kv_1 = cj_0, kv_{j+1} = bd*kv_j + cj_j) and the
                            # remote entering state pre-decayed per block
                            # (kve_j = bd^j * kv_enter). With these ready, every
                            # block's output is ONE psum accumulation group.
                            kvloc, kves = {}, {}
                            for h in heads:
                                cjs = []
                                for j in range(NBLK - 1):
                                    cj = mp.tile(
                                        [128, 128], BF16, tag=f"cj{j}{h % 2}",
                                        name=f"cj{h}_{j}",
                                    )
                                    nc.sync.dma_start(
                                        cj[:],
                                        c_d[(h * NBLK + j) * 128
                                            : (h * NBLK + j + 1) * 128, :],
                                    )
                                    cjs.append(cj)
                                kv2 = mp.tile(
                                    [128, 128], BF16, tag=f"kva{h % 2}0",
                                    name=f"kvn{h}_2",
                                )
                                nc.vector.scalar_tensor_tensor(
                                    kv2[:], cjs[0][:], bd[h], cjs[1][:],
                                    OP.mult, OP.add,
                                )
                                kv3 = mp.tile(
                                    [128, 128], BF16, tag=f"kva{h % 2}1",
                                    name=f"kvn{h}_3",
                                )
                                nc.vector.scalar_tensor_tensor(
                                    kv3[:], kv2[:], bd[h], cjs[2][:],
                                    OP.mult, OP.add,
                                )
                                kvloc[h] = [None, cjs[0], kv2, kv3]
                                kl = [entering_kv(h)]
                                for j in range(1, NBLK):
                                    kd = mp.tile(
                                        [128, 128], BF16, tag=f"kve{h % 4}{j}",
                                        bufs=1, name=f"kve{h}_{j}",
                                    )
                                    nc.vector.tensor_scalar_mul(
                                        kd[:], kl[-1][:], bd[h]
                                    )
                                    kl.append(kd)
                                kves[h] = kl
                            # ---- B1: per head, per block, one accumulation
                            # group: intra(2 MM) + local kv + remote kv, then a
                            # single scalar-engine copy to outT.
                            for h in heads:
                                mq = mp.tile([128, 3 * BLK], BF16, tag=f"mq{h % 2}")
                                nc.sync.dma_start(
                                    mq[:], maskq_d[h * 128 : (h + 1) * 128, :]
                                )
                                qdb = mq[:, 2 * BLK : 3 * BLK]
                                qts = mp.tile([128, T], BF16, tag=f"qts{h % 4}",
                                              bufs=1, name=f"qts{h}")
                                for j in range(NBLK):
                                    nc.vector.tensor_mul(
                                        qts[:, j * BLK : (j + 1) * BLK],
                                        qT[h][:, j * BLK : (j + 1) * BLK],
                                        qdb,
                                    )

                                def emit_qk(j):
                                    # both 128-row halves of k.T @ q_block in ONE
                                    # psum bank (one accumulation group, disjoint
                                    # halves), then ONE [128,512] mask multiply.
                                    col = j * BLK
                                    qkP = psB.tile(
                                        [128, 2 * BLK], FP32, tag="qkP", bufs=3,
                                        name=f"qkP_{h}_{j}",
                                    )
                                    nc.tensor.matmul(
                                        qkP[:, 0:BLK], kT[h][:, col : col + 128],
                                        qT[h][:, col : col + BLK],
                                        start=True, stop=False,
                                    )
                                    nc.tensor.matmul(
                                        qkP[:, BLK : 2 * BLK],
                                        kT[h][:, col + 128 : col + BLK],
                                        qT[h][:, col : col + BLK],
                                        start=False, stop=True,
                                    )
                                    qmP = wp.tile(
                                        [128, 2 * BLK], BF16, tag="qmP", bufs=3,
                                        name=f"qmP_{h}_{j}",
                                    )
                                    nc.vector.tensor_mul(
                                        qmP[:], qkP[:], mq[:, 0 : 2 * BLK]
                                    )
                                    return qmP[:, 0:BLK], qmP[:, BLK : 2 * BLK]

                                qms = {0: emit_qk(0)}
                                for j in range(NBLK):
                                    col = j * BLK
                                    if j + 1 < NBLK:
                                        qms[j + 1] = emit_qk(j + 1)
                                    qm0, qm1 = qms.pop(j)
                                    po = psB.tile(
                                        [128, BLK], FP32, tag="poX", bufs=3,
                                        name=f"poI_{h}_{j}",
                                    )
                                    nc.tensor.matmul(
                                        po[:], vn[2 * j][:, h * 128 : (h + 1) * 128],
                                        qm0[:], start=True, stop=False,
                                    )
                                    nc.tensor.matmul(
                                        po[:],
                                        vn[2 * j + 1][:, h * 128 : (h + 1) * 128],
                                        qm1[:], start=False, stop=False,
                                    )
                                    if j >= 1:
                                        nc.tensor.matmul(
                                            po[:], kvloc[h][j][:],
                                            qts[:, col : col + BLK],
                                            start=False, stop=False,
                                        )
                                    nc.tensor.matmul(
                                        po[:], kves[h][j][:],
                                        qts[:, col : col + BLK],
                                        start=False, stop=True,
                                    )
                                    nc.scalar.activation(
                                        outT[h][:, col : col + BLK], po[:], AF.Copy
                                    )
                                sq = wp.tile([128, T], BF16, tag="big2", bufs=1)
                                nc.vector.tensor_mul(sq[:], outT[h][:], outT[h][:])
                                nc.tensor.matmul(
                                    s0[:], onesb[:], sq[:, 0:512],
                                    start=(h == 0), stop=(h == H - 1),
                                )
                                nc.tensor.matmul(
                                    s1[:], onesb[:], sq[:, 512:1024],
                                    start=(h == 0), stop=(h == H - 1),
                                )

                    # ------------ RMS scale: sc[128, T] = rsqrt(mean + eps)
                    with tc.tile_pool(name="psC", bufs=1, space="PSUM") as psC:
                        sns = []
                        for nn, sacc in enumerate((s0, s1)):
                            sn = wp.tile([1, 512], FP32, tag=f"rms{nn}", bufs=1)
                            nc.scalar.activation(
                                sn[:], sacc[:], AF.Copy,
                                scale=1.0 / (H * D), bias=EPS,
                            )
                            sns.append(sn)
                        for nn in range(2):
                            psc = psC.tile([128, 512], FP32, tag=f"bc{nn}")
                            nc.tensor.matmul(
                                psc[:], onesf[:],
                                sns[nn][:],
                                start=True, stop=True,
                            )
                            stf = wp.tile([128, 512], FP32, tag="stf")
                            nc.scalar.activation(stf[:], psc[:], AF.Sqrt)
                            with nc.allow_low_precision(
                                reason="rms scale rounded to bf16, matches bf16 mul path"
                            ):
                                nc.vector.reciprocal(
                                    sc[:, nn * 512 : (nn + 1) * 512], stf[:]
                                )

            # ---------------- PHASE D: gate + aT, then out projection
            with tc.tile_pool(name="xTD", bufs=1) as xp2, tc.tile_pool(
                name="wo", bufs=1
            ) as wop, tc.tile_pool(name="dwork", bufs=1) as dwp, tc.tile_pool(
                name="wgp", bufs=2
            ) as wgp:
                def load_wg(hgi):
                    tiles = []
                    for kk in range(KC):
                        wt = wgp.tile([128, 512], BF16, tag=f"wg{kk % 8}",
                                      name=f"wg{hgi}_{kk}")
                        eng = nc.scalar if kk % 2 == 0 else nc.sync
                        eng.dma_start(
                            wt[:],
                            wgT_d[
                                kk * 128 : (kk + 1) * 128,
                                hgi * 512 : (hgi + 1) * 512,
                            ],
                        )
                        tiles.append(wt)
                    return tiles

                xk2 = []
                for kk in range(KC):
                    xt = xp2.tile([128, T], BF16, tag=f"y{kk}")
                    nc.scalar.dma_start(xt[:], xT_d[kk * 128 : (kk + 1) * 128, :])
                    xk2.append(xt)
                wg0_tiles = load_wg(0)
                wo_t = []
                for kk in range(KC):
                    wt = wop.tile([128, HID], BF16, tag=f"wo{kk}", name=f"wot{kk}")
                    nc.sync.dma_start(
                        wt[:], woT_d[kk * 128 : (kk + 1) * 128, :]
                    )
                    wo_t.append(wt)

                with tc.tile_pool(
                    name="psG", bufs=1, space="PSUM"
                ) as psG:
                    for hgi in range(4):
                        wg_tiles = wg0_tiles if hgi == 0 else load_wg(hgi)
                        pgs = {}
                        for half in range(2):
                            for i in range(4):
                                pgs[(half, i)] = psG.tile(
                                    [128, 512], FP32, tag=f"pg{half}_{i}",
                                    name=f"pg{hgi}_{half}_{i}",
                                )
                            for kk in range(KC):
                                for i in range(4):
                                    nc.tensor.matmul(
                                        pgs[(half, i)][:],
                                        wg_tiles[kk][:, i * 128 : (i + 1) * 128],
                                        xk2[kk][:, half * 512 : (half + 1) * 512],
                                        start=(kk == 0), stop=(kk == KC - 1),
                                    )
                        for i in range(4):
                            h = hgi * 4 + i
                            aT = pp.tile([128, T], BF16, tag=f"o{h}", name=f"aT{h}")
                            tmps = []
                            for half in range(2):
                                cs = slice(half * 512, (half + 1) * 512)
                                gt = dwp.tile([128, 512], BF16, tag=f"gt{half}")
                                nc.scalar.activation(
                                    gt[:], pgs[(half, i)][:], AF.Sigmoid
                                )
                                tmp = dwp.tile([128, 512], BF16, tag=f"gm{half}")
                                nc.vector.tensor_mul(tmp[:], outT[h][:, cs], gt[:])
                                tmps.append(tmp)
                            for half in range(2):
                                cs = slice(half * 512, (half + 1) * 512)
                                nc.vector.tensor_mul(
                                    aT[:, cs], tmps[half][:], sc[:, cs]
                                )
                            outT[h] = aT

                # ---------------- out projection: tt-major, wo resident
                with tc.tile_pool(name="psF", bufs=2, space="PSUM") as psF:
                    for tt in range(T // 128):
                        pf = [
                            psF.tile([128, 512], FP32, tag=f"f{oc}",
                                     name=f"pf{tt}_{oc}")
                            for oc in range(4)
                        ]
                        for kk in range(KC):
                            lhs = outT[kk][:, tt * 128 : (tt + 1) * 128]
                            for oc in range(4):
                                nc.tensor.matmul(
                                    pf[oc][:], lhs,
                                    wo_t[kk][:, oc * 512 : (oc + 1) * 512],
                                    start=(kk == 0), stop=(kk == KC - 1),
                                )
                        ob = dwp.tile([128, HID], BF16, tag=f"ob{tt % 2}")
                        for oc in range(4):
                            if oc % 2 == 0:
                                with nc.allow_low_precision(
                                    reason="bf16 output store, host converts to fp32"
                                ):
                                    nc.vector.tensor_copy(
                                        ob[:, oc * 512 : (oc + 1) * 512], pf[oc][:]
                                    )
                            else:
                                nc.scalar.activation(
                                    ob[:, oc * 512 : (oc + 1) * 512], pf[oc][:],
                                    AF.Copy,
                                )
                        nc.sync.dma_start(
                            out_d[tt * 128 : (tt + 1) * 128, :], ob[:]
                        )

    nc.compile()
    return nc


def _prep_inputs(x, slope_rate, Wqkv, Wg, norm_w, Wo):
    s = np.asarray(slope_rate, np.float32).reshape(H)
    bd = [float(np.exp(-256.0 * float(sh))) for sh in s]

    # Wqkv rows: head h occupies rows [h*384, (h+1)*384) = q(128) k(128) v(128)
    Wf = np.asarray(Wqkv, np.float32).reshape(H, 3, D, HID)
    wqkT = np.ascontiguousarray(Wf[:, 0:2].reshape(H * 2 * D, HID).T).astype(bf16)
    wvT = np.ascontiguousarray(Wf[:, 2].reshape(H * D, HID).T).astype(bf16)
    wgT = np.ascontiguousarray(np.asarray(Wg, np.float32).T).astype(bf16)
    woT = np.ascontiguousarray(
        np.asarray(Wo, np.float32).T
        * np.asarray(norm_w, np.float32).reshape(H * D, 1)
    ).astype(bf16)

    t_idx = np.arange(BLK, dtype=np.float32)
    maskq = np.zeros((H, 128, 3 * BLK), np.float32)
    kdec = np.zeros((128, H * 2), np.float32)
    for h in range(H):
        mm, nn = np.meshgrid(t_idx, t_idx, indexing="ij")  # mm query, nn key
        mh = np.where(mm >= nn, np.exp(-s[h] * np.maximum(mm - nn, 0.0)), 0.0)
        mt = mh.T  # (n, m)
        maskq[h, :, 0:BLK] = mt[:128]
        maskq[h, :, BLK : 2 * BLK] = mt[128:]
        maskq[h, :, 2 * BLK : 3 * BLK] = np.exp(-s[h] * (t_idx + 1.0))[None, :]
        kd = np.exp(-s[h] * (255.0 - t_idx))
        kdec[:, 2 * h] = kd[:128]
        kdec[:, 2 * h + 1] = kd[128:]
    maskq_a = maskq.reshape(H * 128, 3 * BLK).astype(bf16)

    common = dict(
        wqkT=wqkT, wvT=wvT, wgT=wgT, woT=woT, maskq=maskq_a,
        kdec=np.ascontiguousarray(kdec),
        ident=np.eye(128, dtype=bf16),
        ones=np.ones((128, 1), dtype=bf16),
        onesf=np.ones((1, 128), np.float32),
    )

    x = np.asarray(x, np.float32)
    in_maps = []
    for c in range(NC):
        beta, q = c // 4, c % 4
        xs = x[beta, q * T : (q + 1) * T, :]  # (T, HID)
        xT = np.ascontiguousarray(xs.T).astype(bf16)
        wrep = np.zeros((128, H, 4), np.float32)
        for h in range(H):
            for p in range(4):
                if p < q:
                    wrep[:, h, p] = bd[h] ** (NBLK * (q - 1 - p))
        in_maps.append(
            dict(common, xT=xT, wrep=np.ascontiguousarray(wrep.reshape(128, H * 4)))
        )
    return bd, in_maps


_CACHE = {}


def _get_nc(bd):
    key = tuple(bd)
    if key not in _CACHE:
        _CACHE[key] = _build(bd)
    return _CACHE[key]


def kernel(x, slope_rate, Wqkv, Wg, norm_w, Wo, _trace=False, _trace_kwargs=None):
    bd, in_maps = _prep_inputs(x, slope_rate, Wqkv, Wg, norm_w, Wo)
    nc = _get_nc(bd)
    res = run_bass_kernel_spmd(
        nc, in_maps, core_ids=list(range(NC)), trace=_trace,
        **(_trace_kwargs or {}),
    )
    out = np.zeros((B, N, HID), np.float32)
    for c in range(NC):
        beta, q = c // 4, c % 4
        out[beta, q * T : (q + 1) * T, :] = np.asarray(
            res.results[c]["out"], np.float32
        )
    kernel._last_result = res
    return out


# revision 57
# speedup vs baseline: 1.0288x; 1.0288x over previous
"""MiniMaxText01 lightning-attention kernel for 8 TRN2 NeuronCores.

Sharding: 8 cores = 2 batches x 4 sequence quarters (token-parallel).
Each core runs the whole pipeline (qkv proj -> decay block scan -> RMSNorm
-> gate -> out proj) for its 1024 tokens; the only cross-core data is the
kv-state prefix, exchanged via a 1MB AllGather of per-core decayed kv
contributions within each batch's 4-core group.

All matmuls in bf16 (fp32 PSUM accumulation); RMS scale path in fp32.
v2: weight DMAs split across sync+scalar HWDGE queues, RMS sums fused
into the block scan, gate GEMM pipelined across 4-head groups, out-proj
with fully-resident Wo and 1MB stores.
"""

import sys

sys.path.insert(0, "/opt/trn_rl_repo")

import ml_dtypes
import numpy as np

import types

try:
    import antenv.axon_hooks  # noqa: F401
except ImportError:
    try:
        import antenv
        from trn_agent_boot.trn_boot import _ntff_profile_via_ctypes

        _m = types.ModuleType("antenv.axon_hooks")
        _m._hook = _ntff_profile_via_ctypes("/opt/axon/libaxon_pjrt.so")
        _m.get_axon_ntff_profile_hook = lambda: _m._hook
        _m.set_axon_ntff_profile_hook = lambda h: setattr(_m, "_hook", h)
        sys.modules["antenv.axon_hooks"] = _m
        antenv.axon_hooks = _m
    except Exception:
        pass

import concourse.bass as bass
import concourse.mybir as mybir
from concourse import bacc
from concourse.tile import TileContext
from concourse.bass_utils import run_bass_kernel_spmd

BF16 = mybir.dt.bfloat16
FP32 = mybir.dt.float32
AF = mybir.ActivationFunctionType
OP = mybir.AluOpType
bf16 = ml_dtypes.bfloat16

B, N, HID = 2, 4096, 2048
H, D, BLK = 16, 128, 256
T = 1024          # tokens per core
NBLK = T // BLK   # 4 local blocks
KC = HID // 128   # 16 contraction chunks
NC = 8
EPS = float(np.finfo(np.float32).eps)


def _build(bd):
    """Build the SPMD bass program. bd: (16,) python floats exp(-256*s_h)."""
    nc = bacc.Bacc("TRN2", target_bir_lowering=False, debug=False, num_devices=NC)

    xT_d = nc.dram_tensor("xT", [HID, T], BF16, kind="ExternalInput")
    wqk_d = nc.dram_tensor("wqkT", [HID, H * 2 * D], BF16, kind="ExternalInput")
    wv_d = nc.dram_tensor("wvT", [HID, H * D], BF16, kind="ExternalInput")
    c_d = nc.dram_tensor("cscratch", [H * NBLK * 128, 128], BF16, kind="Internal")
    wgT_d = nc.dram_tensor("wgT", [HID, H * D], BF16, kind="ExternalInput")
    woT_d = nc.dram_tensor("woT", [H * D, HID], BF16, kind="ExternalInput")
    maskq_d = nc.dram_tensor("maskq", [H * 128, 3 * BLK], BF16, kind="ExternalInput")
    kdec_d = nc.dram_tensor("kdec", [128, H * 2], FP32, kind="ExternalInput")
    wrep_d = nc.dram_tensor("wrep", [128, H * 4], FP32, kind="ExternalInput")
    ident_d = nc.dram_tensor("ident", [128, 128], BF16, kind="ExternalInput")
    ones_d = nc.dram_tensor("ones", [128, 1], BF16, kind="ExternalInput")
    onesf_d = nc.dram_tensor("onesf", [1, 128], FP32, kind="ExternalInput")
    onescf_d = nc.dram_tensor("onescf", [128, 1], FP32, kind="ExternalInput")
    out_d = nc.dram_tensor("out", [T, HID], BF16, kind="ExternalOutput")
    ccin_g = [
        nc.dram_tensor(f"ccin{g}", [4 * 128, 128], FP32, kind="Internal")
        for g in range(4)
    ]
    ccout_g = [
        nc.dram_tensor(f"ccout{g}", [4 * 4 * 128, 128], FP32, kind="Internal")
        for g in range(4)
    ]
    warm_in = nc.dram_tensor("warmin", [128, 128], FP32, kind="Internal")
    warm_out = nc.dram_tensor("warmout", [4 * 128, 128], FP32, kind="Internal")

    with TileContext(nc) as tc:
        with (
            tc.tile_pool(name="const", bufs=1) as cp,
            tc.tile_pool(name="persist", bufs=1) as pp,
            tc.tile_pool(name="work", bufs=2) as wp,
        ):
            # warm up the collective path so the first real AllGather is hot
            nc.gpsimd.collective_compute(
                "AllGather",
                OP.bypass,
                ins=[warm_in[:]],
                outs=[warm_out[:]],
                replica_groups=[[0, 1, 2, 3], [4, 5, 6, 7]],
            )
            ident = cp.tile([128, 128], BF16)
            nc.scalar.dma_start(ident[:], ident_d[:])
            onesb = cp.tile([128, 1], BF16)
            nc.scalar.dma_start(onesb[:], ones_d[:])
            kdec = cp.tile([128, H * 2], FP32)
            nc.scalar.dma_start(kdec[:], kdec_d[:])
            wrep = cp.tile([128, H * 4], FP32)
            nc.scalar.dma_start(wrep[:], wrep_d[:])
            onesf = cp.tile([1, 128], FP32)
            nc.scalar.dma_start(onesf[:], onesf_d[:])
            onescf = cp.tile([128, 1], FP32)
            nc.scalar.dma_start(onescf[:], onescf_d[:])
            sc = cp.tile([128, T], BF16)

            qT, kT, outT = [], [], []
            for h in range(H):
                outT.append(pp.tile([128, T], BF16, tag=f"o{h}", name=f"outT{h}"))

            # ---------------- PASS A: qkv projection + kv contributions
            with tc.tile_pool(name="qkv", bufs=1) as qp:
                with tc.tile_pool(name="xTA", bufs=1) as xp:
                    xk = []
                    for kk in range(KC):
                        xt = xp.tile([128, T], BF16, tag=f"x{kk}")
                        nc.gpsimd.dma_start(xt[:], xT_d[kk * 128 : (kk + 1) * 128, :])
                        xk.append(xt)
                    # v projection, token-partition orientation, one 512-col
                    # group at a time (keeps resident v-weights at 1MB)
                    vn = [
                        qp.tile([128, H * D], BF16, tag=f"vn{tt}", name=f"vn{tt}")
                        for tt in range(T // 128)
                    ]
                    with tc.tile_pool(name="wvp", bufs=1) as wvpool, \
                         tc.tile_pool(name="psV", bufs=1, space="PSUM") as psV:
                        for oc in range(4):
                            pvs = {}
                            for kkh in range(2):
                                wv_tiles = []
                                for k8 in range(8):
                                    kk = kkh * 8 + k8
                                    wvp = wvpool.tile(
                                        [128, 512], BF16, tag=f"wvk{kk}",
                                        name=f"wvk{kk}_{oc}",
                                    )
                                    eng = nc.sync if k8 % 2 == 0 else nc.scalar
                                    eng.dma_start(
                                        wvp[:],
                                        wv_d[
                                            kk * 128 : (kk + 1) * 128,
                                            oc * 512 : (oc + 1) * 512,
                                        ],
                                    )
                                    wv_tiles.append(wvp)
                                for tt in range(T // 128):
                                    if kkh == 0:
                                        pvs[tt] = psV.tile(
                                            [128, 512], FP32, tag=f"vo{tt}",
                                            name=f"pv{oc}_{tt}",
                                        )
                                    pv = pvs[tt]
                                    for k8 in range(8):
                                        kk = kkh * 8 + k8
                                        nc.tensor.matmul(
                                            pv[:],
                                            xk[kk][:, tt * 128 : (tt + 1) * 128],
                                            wv_tiles[k8][:],
                                            start=(kk == 0), stop=(kk == KC - 1),
                                        )
                                    if kkh == 1:
                                        nc.scalar.activation(
                                            vn[tt][:, oc * 512 : (oc + 1) * 512],
                                            pv[:], AF.Silu,
                                        )
                    with tc.tile_pool(name="wqkp", bufs=1) as wqkp, tc.tile_pool(
                        name="psA", bufs=1, space="PSUM"
                    ) as psA, tc.tile_pool(name="psT", bufs=1, space="PSUM") as psT:
                        for h in range(H):
                            q_t = qp.tile([128, T], BF16, tag=f"q{h}", name=f"q{h}")
                            k_t = qp.tile([128, T], BF16, tag=f"k{h}", name=f"k{h}")
                            qT.append(q_t)
                            kT.append(k_t)
                            ps = [
                                psA.tile([128, 512], FP32, tag=f"pj{i}",
                                         name=f"pj{h}_{i}")
                                for i in range(4)
                            ]
                            wts = []
                            for kk in range(KC):
                                wt = wqkp.tile([128, 2 * D], BF16, tag=f"w{kk}",
                                               name=f"w{h}_{kk}")
                                eng = nc.sync if kk % 2 == 0 else nc.scalar
                                eng.dma_start(
                                    wt[:],
                                    wqk_d[
                                        kk * 128 : (kk + 1) * 128,
                                        h * 2 * D : (h + 1) * 2 * D,
                                    ],
                                )
                                wts.append(wt)
                            for kk in range(KC):
                                for si in range(2):
                                    lhs = wts[kk][:, si * 128 : (si + 1) * 128]
                                    for nn in range(2):
                                        nc.tensor.matmul(
                                            ps[2 * si + nn][:],
                                            lhs,
                                            xk[kk][:, nn * 512 : (nn + 1) * 512],
                                            start=(kk == 0),
                                            stop=(kk == KC - 1),
                                        )
                            for si, dst in enumerate((q_t, k_t)):
                                for nn in range(2):
                                    nc.scalar.activation(
                                        dst[:, nn * 512 : (nn + 1) * 512],
                                        ps[2 * si + nn][:],
                                        AF.Silu,
                                    )
                            # kv contributions of local blocks, decayed to core end
                            totB = wp.tile([128, 128], FP32, tag="totB")
                            for j in range(NBLK):
                                csum = psT.tile([128, 128], FP32, tag="Cp")
                                for hf in range(2):
                                    col = j * BLK + hf * 128
                                    tt = j * 2 + hf
                                    pt = psT.tile([128, 128], BF16, tag="tr")
                                    nc.tensor.transpose(
                                        pt[:], k_t[:, col : col + 128], ident[:]
                                    )
                                    ks = wp.tile([128, 128], BF16, tag="ks")
                                    nc.vector.tensor_scalar_mul(
                                        ks[:], pt[:],
                                        kdec[:, 2 * h + hf : 2 * h + hf + 1],
                                    )
                                    nc.tensor.matmul(
                                        csum[:], ks[:],
                                        vn[tt][:, h * 128 : (h + 1) * 128],
                                        start=(hf == 0), stop=(hf == 1),
                                    )
                                if j < NBLK - 1:
                                    cb = wp.tile([128, 128], BF16, tag="cb",
                                                 bufs=6)
                                    nc.vector.tensor_copy(cb[:], csum[:])
                                    nc.gpsimd.dma_start(
                                        c_d[(h * NBLK + j) * 128
                                            : (h * NBLK + j + 1) * 128, :],
                                        cb[:],
                                    )
                                w = bd[h] ** (NBLK - 1 - j)
                                if j == 0:
                                    nc.vector.tensor_scalar_mul(totB[:], csum[:], w)
                                else:
                                    nc.vector.scalar_tensor_tensor(
                                        totB[:], csum[:], w, totB[:], OP.mult, OP.add
                                    )
                            g, hg = h // 4, h % 4
                            nc.sync.dma_start(
                                ccin_g[g][hg * 128 : (hg + 1) * 128, :], totB[:]
                            )
                            if hg == 3:
                                nc.gpsimd.collective_compute(
                                    "AllGather",
                                    OP.bypass,
                                    ins=[ccin_g[g][:]],
                                    outs=[ccout_g[g][:]],
                                    replica_groups=[[0, 1, 2, 3], [4, 5, 6, 7]],
                                )
                            if h == H - 1:
                                # WAW anchor: bind the ccout-load tile tags to
                                # end-of-pass-A work so the scheduler cannot
                                # queue those (collective-gated) DMAs ahead of
                                # the remaining c_d stores on the same queue.
                                for tgi in range(2):
                                    gw = wp.tile([128, 128], FP32, tag=f"g{tgi}")
                                    nc.vector.tensor_copy(gw[:], totB[:])

                def entering_kv(h):
                    """kv state entering this core for head h, from the AllGather."""
                    ent = wp.tile([128, 128], FP32, tag="ent")
                    gi, hg = h // 4, h % 4
                    for p in range(4):
                        g = wp.tile([128, 128], FP32, tag=f"g{p % 2}")
                        nc.gpsimd.dma_start(
                            g[:],
                            ccout_g[gi][
                                (p * 4 + hg) * 128 : (p * 4 + hg + 1) * 128, :
                            ],
                        )
                        if p == 0:
                            nc.vector.tensor_scalar_mul(
                                ent[:], g[:], wrep[:, 4 * h : 4 * h + 1]
                            )
                        else:
                            nc.vector.scalar_tensor_tensor(
                                ent[:], g[:], wrep[:, 4 * h + p : 4 * h + p + 1],
                                ent[:], OP.mult, OP.add,
                            )
                    kv = pp.tile([128, 128], BF16, tag=f"kv{h}b")
                    nc.vector.tensor_copy(kv[:], ent[:])
                    return kv

                # ---------------- PASS B: decay block scan (+ fused RMS sums)
                # B1 (per head): dense intra-block attention, no serial deps.
                # B2 (per 4-head group): inter-block kv chains, 4 chains
                # interleaved so the PE never waits on one chain's latency.
                if True:
                    sqacc = wp.tile([128, T], FP32, tag="sqa", bufs=1)
                    with tc.tile_pool(name="psB", bufs=2, space="PSUM") as psB, \
                         tc.tile_pool(name="mp", bufs=2) as mp:
                        for hgi in range(4):
                            heads = range(4 * hgi, 4 * hgi + 4)
                            # ---- vector-side prep (off the PE critical path):
                            # local kv states from the c_d block contributions
                            # (kv_1 = cj_0, kv_{j+1} = bd*kv_j + cj_j) and the
                            # remote entering state pre-decayed per block
                            # (kve_j = bd^j * kv_enter). With these ready, every
                            # block's output is ONE psum accumulation group.
                            kvloc, kves = {}, {}
                            for h in heads:
                                cjs = []
                                for j in range(NBLK - 1):
                                    cj = mp.tile(
                                        [128, 128], BF16, tag=f"cj{j}{h % 2}",
                                        name=f"cj{h}_{j}",
                                    )
                                    nc.sync.dma_start(
                                        cj[:],
                                        c_d[(h * NBLK + j) * 128
                                            : (h * NBLK + j + 1) * 128, :],
                                    )
                                    cjs.append(cj)
                                kv2 = mp.tile(
                                    [128, 128], BF16, tag=f"kva{h % 2}0",
                                    name=f"kvn{h}_2",
                                )
                                nc.vector.scalar_tensor_tensor(
                                    kv2[:], cjs[0][:], bd[h], cjs[1][:],
                                    OP.mult, OP.add,
                                )
                                kv3 = mp.tile(
                                    [128, 128], BF16, tag=f"kva{h % 2}1",
                                    name=f"kvn{h}_3",
                                )
                                nc.vector.scalar_tensor_tensor(
                                    kv3[:], kv2[:], bd[h], cjs[2][:],
                                    OP.mult, OP.add,
                                )
                                kvloc[h] = [None, cjs[0], kv2, kv3]
                                kl = [entering_kv(h)]
                                for j in range(1, NBLK):
                                    kd = mp.tile(
                                        [128, 128], BF16, tag=f"kve{h % 4}{j}",
                                        bufs=1, name=f"kve{h}_{j}",
                                    )
                                    nc.vector.tensor_scalar_mul(
                                        kd[:], kl[-1][:], bd[h]
                                    )
                                    kl.append(kd)
                                kves[h] = kl
                            # ---- B1: per head, per block, one accumulation
                            # group: intra(2 MM) + local kv + remote kv, then a
                            # single scalar-engine copy to outT.
                            for h in heads:
                                mq = mp.tile([128, 3 * BLK], BF16, tag=f"mq{h % 2}")
                                nc.sync.dma_start(
                                    mq[:], maskq_d[h * 128 : (h + 1) * 128, :]
                                )
                                qdb = mq[:, 2 * BLK : 3 * BLK]
                                qts = mp.tile([128, T], BF16, tag=f"qts{h % 4}",
                                              bufs=1, name=f"qts{h}")
                                for j in range(NBLK):
                                    nc.vector.tensor_mul(
                                        qts[:, j * BLK : (j + 1) * BLK],
                                        qT[h][:, j * BLK : (j + 1) * BLK],
                                        qdb,
                                    )

                                def emit_qk(j):
                                    # both 128-row halves of k.T @ q_block in ONE
                                    # psum bank (one accumulation group, disjoint
                                    # halves), then ONE [128,512] mask multiply.
                                    col = j * BLK
                                    qkP = psB.tile(
                                        [128, 2 * BLK], FP32, tag="qkP", bufs=3,
                                        name=f"qkP_{h}_{j}",
                                    )
                                    nc.tensor.matmul(
                                        qkP[:, 0:BLK], kT[h][:, col : col + 128],
                                        qT[h][:, col : col + BLK],
                                        start=True, stop=False,
                                    )
                                    nc.tensor.matmul(
                                        qkP[:, BLK : 2 * BLK],
                                        kT[h][:, col + 128 : col + BLK],
                                        qT[h][:, col : col + BLK],
                                        start=False, stop=True,
                                    )
                                    qmP = wp.tile(
                                        [128, 2 * BLK], BF16, tag="qmP", bufs=3,
                                        name=f"qmP_{h}_{j}",
                                    )
                                    nc.vector.tensor_mul(
                                        qmP[:], qkP[:], mq[:, 0 : 2 * BLK]
                                    )
                                    return qmP[:, 0:BLK], qmP[:, BLK : 2 * BLK]

                                qms = {0: emit_qk(0)}
                                for j in range(NBLK):
                                    col = j * BLK
                                    if j + 1 < NBLK:
                                        qms[j + 1] = emit_qk(j + 1)
                                    qm0, qm1 = qms.pop(j)
                                    po = psB.tile(
                                        [128, BLK], FP32, tag="poX", bufs=4,
                                        name=f"poI_{h}_{j}",
                                    )
                                    nc.tensor.matmul(
                                        po[:], vn[2 * j][:, h * 128 : (h + 1) * 128],
                                        qm0[:], start=True, stop=False,
                                    )
                                    nc.tensor.matmul(
                                        po[:],
                                        vn[2 * j + 1][:, h * 128 : (h + 1) * 128],
                                        qm1[:], start=False, stop=False,
                                    )
                                    if j >= 1:
                                        nc.tensor.matmul(
                                            po[:], kvloc[h][j][:],
                                            qts[:, col : col + BLK],
                                            start=False, stop=False,
                                        )
                                    nc.tensor.matmul(
                                        po[:], kves[h][j][:],
                                        qts[:, col : col + BLK],
                                        start=False, stop=True,
                                    )
                                    nc.scalar.activation(
                                        outT[h][:, col : col + BLK], po[:], AF.Copy
                                    )
                                # accumulate sum-of-squares across heads on the
                                # vector engine (no PE involvement until the
                                # final partition reduction after pass B)
                                if h == 0:
                                    nc.vector.tensor_mul(
                                        sqacc[:], outT[h][:], outT[h][:]
                                    )
                                else:
                                    sq = wp.tile([128, T], BF16, tag="big2",
                                                 bufs=1)
                                    nc.vector.tensor_mul(
                                        sq[:], outT[h][:], outT[h][:]
                                    )
                                    nc.vector.scalar_tensor_tensor(
                                        sqacc[:], sq[:], 1.0, sqacc[:],
                                        OP.mult, OP.add,
                                    )

                    # ------------ RMS scale: sc[128, T] = rsqrt(mean + eps)
                    # partition-reduce sqacc (fp32 matmul with ones), broadcast
                    # back to 128 partitions, then sqrt+reciprocal.
                    with tc.tile_pool(name="psC", bufs=1, space="PSUM") as psC:
                        sns = []
                        for nn in range(2):
                            ssum = psC.tile([1, 512], FP32, tag=f"s{nn}")
                            nc.tensor.matmul(
                                ssum[:], onescf[:],
                                sqacc[:, nn * 512 : (nn + 1) * 512],
                                start=True, stop=True,
                            )
                            sn = wp.tile([1, 512], FP32, tag=f"rms{nn}", bufs=1)
                            nc.scalar.activation(
                                sn[:], ssum[:], AF.Copy,
                                scale=1.0 / (H * D), bias=EPS,
                            )
                            sns.append(sn)
                        for nn in range(2):
                            psc = psC.tile([128, 512], FP32, tag=f"bc{nn}")
                            nc.tensor.matmul(
                                psc[:], onesf[:],
                                sns[nn][:],
                                start=True, stop=True,
                            )
                            stf = wp.tile([128, 512], FP32, tag="stf")
                            nc.scalar.activation(stf[:], psc[:], AF.Sqrt)
                            with nc.allow_low_precision(
                                reason="rms scale rounded to bf16, matches bf16 mul path"
                            ):
                                nc.vector.reciprocal(
                                    sc[:, nn * 512 : (nn + 1) * 512], stf[:]
                                )

            # ---------------- PHASE D: gate + aT, then out projection
            with tc.tile_pool(name="xTD", bufs=1) as xp2, tc.tile_pool(
                name="wo", bufs=1
            ) as wop, tc.tile_pool(name="dwork", bufs=1) as dwp, tc.tile_pool(
                name="wgp", bufs=2
            ) as wgp:
                def load_wg(hgi):
                    tiles = []
                    for kk in range(KC):
                        wt = wgp.tile([128, 512], BF16, tag=f"wg{kk % 8}",
                                      name=f"wg{hgi}_{kk}")
                        eng = nc.scalar if kk % 2 == 0 else nc.sync
                        eng.dma_start(
                            wt[:],
                            wgT_d[
                                kk * 128 : (kk + 1) * 128,
                                hgi * 512 : (hgi + 1) * 512,
                            ],
                        )
                        tiles.append(wt)
                    return tiles

                xk2 = []
                for kk in range(KC):
                    xt = xp2.tile([128, T], BF16, tag=f"y{kk}")
                    eng = nc.scalar if kk % 2 == 0 else nc.sync
                    eng.dma_start(xt[:], xT_d[kk * 128 : (kk + 1) * 128, :])
                    xk2.append(xt)
                wg0_tiles = load_wg(0)
                wo_t = []
                for kk in range(KC):
                    wt = wop.tile([128, HID], BF16, tag=f"wo{kk}", name=f"wot{kk}")
                    nc.sync.dma_start(
                        wt[:], woT_d[kk * 128 : (kk + 1) * 128, :]
                    )
                    wo_t.append(wt)

                with tc.tile_pool(
                    name="psG", bufs=1, space="PSUM"
                ) as psG:
                    for hgi in range(4):
                        wg_tiles = wg0_tiles if hgi == 0 else load_wg(hgi)
                        pgs = {}
                        for half in range(2):
                            for i in range(4):
                                pgs[(half, i)] = psG.tile(
                                    [128, 512], FP32, tag=f"pg{half}_{i}",
                                    name=f"pg{hgi}_{half}_{i}",
                                )
                            for kk in range(KC):
                                for i in range(4):
                                    nc.tensor.matmul(
                                        pgs[(half, i)][:],
                                        wg_tiles[kk][:, i * 128 : (i + 1) * 128],
                                        xk2[kk][:, half * 512 : (half + 1) * 512],
                                        start=(kk == 0), stop=(kk == KC - 1),
                                    )
                        for i in range(4):
                            h = hgi * 4 + i
                            aT = pp.tile([128, T], BF16, tag=f"o{h}", name=f"aT{h}")
                            tmps = []
                            for half in range(2):
                                cs = slice(half * 512, (half + 1) * 512)
                                gt = dwp.tile([128, 512], BF16, tag=f"gt{half}")
                                nc.scalar.activation(
                                    gt[:], pgs[(half, i)][:], AF.Sigmoid
                                )
                                tmp = dwp.tile([128, 512], BF16, tag=f"gm{half}")
                                nc.vector.tensor_mul(tmp[:], outT[h][:, cs], gt[:])
                                tmps.append(tmp)
                            for half in range(2):
                                cs = slice(half * 512, (half + 1) * 512)
                                nc.vector.tensor_mul(
                                    aT[:, cs], tmps[half][:], sc[:, cs]
                                )
                            outT[h] = aT

                # ---------------- out projection: tt-major, wo resident
                with tc.tile_pool(name="psF", bufs=2, space="PSUM") as psF:
                    for tt in range(T // 128):
                        pf = [
                            psF.tile([128, 512], FP32, tag=f"f{oc}",
                                     name=f"pf{tt}_{oc}")
                            for oc in range(4)
                        ]
                        for kk in range(KC):
                            lhs = outT[kk][:, tt * 128 : (tt + 1) * 128]
                            for oc in range(4):
                                nc.tensor.matmul(
                                    pf[oc][:], lhs,
                                    wo_t[kk][:, oc * 512 : (oc + 1) * 512],
                                    start=(kk == 0), stop=(kk == KC - 1),
                                )
                        ob = dwp.tile([128, HID], BF16, tag=f"ob{tt % 2}")
                        for oc in range(4):
                            if oc % 2 == 0:
                                with nc.allow_low_precision(
                                    reason="bf16 output store, host converts to fp32"
                                ):
                                    nc.vector.tensor_copy(
                                        ob[:, oc * 512 : (oc + 1) * 512], pf[oc][:]
                                    )
                            else:
                                nc.scalar.activation(
                                    ob[:, oc * 512 : (oc + 1) * 512], pf[oc][:],
                                    AF.Copy,
                                )
                        nc.sync.dma_start(
                            out_d[tt * 128 : (tt + 1) * 128, :], ob[:]
                        )

    nc.compile()
    return nc


def _prep_inputs(x, slope_rate, Wqkv, Wg, norm_w, Wo):
    s = np.asarray(slope_rate, np.float32).reshape(H)
    bd = [float(np.exp(-256.0 * float(sh))) for sh in s]

    # Wqkv rows: head h occupies rows [h*384, (h+1)*384) = q(128) k(128) v(128)
    Wf = np.asarray(Wqkv, np.float32).reshape(H, 3, D, HID)
    wqkT = np.ascontiguousarray(Wf[:, 0:2].reshape(H * 2 * D, HID).T).astype(bf16)
    wvT = np.ascontiguousarray(Wf[:, 2].reshape(H * D, HID).T).astype(bf16)
    wgT = np.ascontiguousarray(np.asarray(Wg, np.float32).T).astype(bf16)
    woT = np.ascontiguousarray(
        np.asarray(Wo, np.float32).T
        * np.asarray(norm_w, np.float32).reshape(H * D, 1)
    ).astype(bf16)

    t_idx = np.arange(BLK, dtype=np.float32)
    maskq = np.zeros((H, 128, 3 * BLK), np.float32)
    kdec = np.zeros((128, H * 2), np.float32)
    for h in range(H):
        mm, nn = np.meshgrid(t_idx, t_idx, indexing="ij")  # mm query, nn key
        mh = np.where(mm >= nn, np.exp(-s[h] * np.maximum(mm - nn, 0.0)), 0.0)
        mt = mh.T  # (n, m)
        maskq[h, :, 0:BLK] = mt[:128]
        maskq[h, :, BLK : 2 * BLK] = mt[128:]
        maskq[h, :, 2 * BLK : 3 * BLK] = np.exp(-s[h] * (t_idx + 1.0))[None, :]
        kd = np.exp(-s[h] * (255.0 - t_idx))
        kdec[:, 2 * h] = kd[:128]
        kdec[:, 2 * h + 1] = kd[128:]
    maskq_a = maskq.reshape(H * 128, 3 * BLK).astype(bf16)

    common = dict(
        wqkT=wqkT, wvT=wvT, wgT=wgT, woT=woT, maskq=maskq_a,
        kdec=np.ascontiguousarray(kdec),
        ident=np.eye(128, dtype=bf16),
        ones=np.ones((128, 1), dtype=bf16),
        onesf=np.ones((1, 128), np.float32),
        onescf=np.ones((128, 1), np.float32),
    )

    x = np.asarray(x, np.float32)
    in_maps = []
    for c in range(NC):
        beta, q = c // 4, c % 4
        xs = x[beta, q * T : (q + 1) * T, :]  # (T, HID)
        xT = np.ascontiguousarray(xs.T).astype(bf16)
        wrep = np.zeros((128, H, 4), np.float32)
        for h in range(H):
            for p in range(4):
                if p < q:
                    wrep[:, h, p] = bd[h] ** (NBLK * (q - 1 - p))
        in_maps.append(
            dict(common, xT=xT, wrep=np.ascontiguousarray(wrep.reshape(128, H * 4)))
        )
    return bd, in_maps


_CACHE = {}


def _get_nc(bd):
    key = tuple(bd)
    if key not in _CACHE:
        _CACHE[key] = _build(bd)
    return _CACHE[key]


def kernel(x, slope_rate, Wqkv, Wg, norm_w, Wo, _trace=False, _trace_kwargs=None):
    bd, in_maps = _prep_inputs(x, slope_rate, Wqkv, Wg, norm_w, Wo)
    nc = _get_nc(bd)
    res = run_bass_kernel_spmd(
        nc, in_maps, core_ids=list(range(NC)), trace=_trace,
        **(_trace_kwargs or {}),
    )
    out = np.zeros((B, N, HID), np.float32)
    for c in range(NC):
        beta, q = c // 4, c % 4
        out[beta, q * T : (q + 1) * T, :] = np.asarray(
            res.results[c]["out"], np.float32
        )
    kernel._last_result = res
    return out
